# revision 41
# baseline (speedup 1.0000x reference)
"""Multi-head attention (B=4, L=1024, D=1024, H=16, DH=64) on 8 TRN2 NeuronCores.

Sharding: data-parallel over batch (4) x tensor-parallel over heads (2).
Core c = 2*b + t computes, for batch b, heads [t*8, (t+1)*8):
    QT = Wq_t^T X^T, KT = Wk_t^T X^T, V = Y Wv_t        (all bf16 matmuls)
    per head: S^T = K_h Q_h^T; P^T = exp(S^T/8);
              [ctx^T; rowsum] = Vaug_h^T P^T;  ctxn = ctx / rowsum
    O_partial = ctxn^T Wo_t                              (f32, two dt-halves)
Host pre-transposes X/Y, casts to bf16, and sums the four f32 partials
(2 tensor-parallel cores x 2 dt-halves) per batch.

Engines execute their compiled instruction streams in order, so the emission
order is a hand-software-pipelined schedule: every ST (scores) step, whose exp
drain on the scalar engine is slower than the matmuls, is followed by an
independent fill chain (V projection, next d-tile QT/KT, an earlier head's
ctx, or an out-projection partial) so the tensor engine never waits for the
scalar engine to free an ST PSUM tile.

Perf notes (vs the first working version):
  - Input DMA configs are spread across four sequencers (SP/Pool/DVE/ACT);
    a single SP rail configures queues at ~0.6us each, serializing the
    input rollout and starving the PE for the first ~15us.
  - The ones-blocks of Vaug are memset with one strided op (half the data).
  - The first QT/KT drains go to the scalar engine (idle before the exps).
  - Tail: ctx tiles for the last head pair live in the (by then idle) wide
    ST PSUM pool so the out-projection chains get the full 4-slot acc pool;
    tail drains alternate scalar/vector; chain order puts both ic0 ctx
    chains first so their normalize DMA round-trips hide under ic1's PE
    work. Keeping the PE stream dense also holds it at the 2.4GHz p-state
    (it drops to 1.2GHz within ~100ns of going idle).
"""

import numpy as np
import ml_dtypes

import concourse.tile as tile
import concourse.mybir as mybir
from concourse import bacc
from concourse.bass_utils import run_bass_kernel_spmd

B, L, D, U, H = 4, 1024, 1024, 1024, 16
DH = U // H          # 64 head dim
TP = 2               # tensor-parallel ways (heads)
DL = U // TP         # 512 local units
HL = H // TP         # 8 local heads
P = 128              # partitions
NI = 512             # matmul free-dim chunk (one PSUM bank of f32)
CC = D // P          # 8 contraction chunks for projections
DT = DL // P         # 4 local d-tiles
IT = L // P          # 8 i/j tiles
NIC = L // NI        # 2 free chunks of 512
N_CORES = 8

BF16 = mybir.dt.bfloat16
F32 = mybir.dt.float32


def _build_kernel():
    nc = bacc.Bacc(
        "TRN2", target_bir_lowering=False, debug=False, num_devices=N_CORES
    )
    xt = nc.dram_tensor("xt", [D, L], BF16, kind="ExternalInput").ap()
    yt = nc.dram_tensor("yt", [D, L], BF16, kind="ExternalInput").ap()
    # wq/wk arrive dt-major: [DT, P, CC*128] (host pre-arranged) so each
    # dt-block is one contiguous 256KB DMA
    wq = nc.dram_tensor("wq", [DT, P, CC * P], BF16, kind="ExternalInput").ap()
    wk = nc.dram_tensor("wk", [DT, P, CC * P], BF16, kind="ExternalInput").ap()
    wv = nc.dram_tensor("wv", [D, DL], BF16, kind="ExternalInput").ap()
    wo = nc.dram_tensor("wo", [DL, U], BF16, kind="ExternalInput").ap()
    out_a = nc.dram_tensor("out_a", [L, U], BF16, kind="ExternalOutput").ap()
    out_c = nc.dram_tensor("out_c", [L, U], BF16, kind="ExternalOutput").ap()

    with tile.TileContext(nc) as tc:
        _mha_body(tc, out_a, out_c, xt, yt, wq, wk, wv, wo)

    nc.compile()
    return nc


def _mha_body(tc, out_a, out_c, xt, yt, wq, wk, wv, wo, dbg=None):
    nc = tc.nc
    from contextlib import ExitStack

    with ExitStack() as ctx:
        persist = ctx.enter_context(tc.tile_pool(name="persist", bufs=1))
        # P^T tiles are per-pair now; live set = current pair + previous
        # (whose ctx chains consume it)
        pt_pool = ctx.enter_context(tc.tile_pool(name="pt", bufs=2))
        # ST tiles: [P, 1024] f32 = 2 banks each; one per (jt, ic) step
        # holding BOTH heads' 512-blocks, so the two K=64 matmuls land in
        # different banks and stream concurrently on disjoint PE row groups
        ps_wide = ctx.enter_context(tc.tile_pool(name="ps_wide", bufs=2, space="PSUM"))
        # single-bank accumulators (projections, V, ctx, out-proj)
        ps_acc = ctx.enter_context(tc.tile_pool(name="ps_acc", bufs=4, space="PSUM"))
        small = ctx.enter_context(tc.tile_pool(name="small", bufs=4))

        # persistent SBUF tensors
        xt_sb = persist.tile([P, CC, L], BF16, tag="xt")
        yt_sb = persist.tile([P, CC, L], BF16, tag="yt")
        # wq/wk are dt-major (host pre-arranged [DT, P, CC*128]) so the
        # dt0 blocks needed by the first ST land after 0.5MB of weight DMA
        # instead of 2MB
        wq_sb = persist.tile([P, DT, CC * P], BF16, tag="wq")
        wk_sb = persist.tile([P, DT, CC * P], BF16, tag="wk")
        wv_sb = persist.tile([P, CC, DL], BF16, tag="wv")
        wo_sb = persist.tile([P, DT, U], BF16, tag="wo")
        qt_sb = persist.tile([P, DT, L], BF16, tag="qt")
        kt_sb = persist.tile([P, DT, L], BF16, tag="kt")
        # Vaug: per j-chunk, per head a 128-col block; even h: [V_h | ones],
        # odd h: [ones | V_h] (ctx^T lands on the head's own cx partitions)
        va_sb = persist.tile([P, IT, HL * P], BF16, tag="va")
        cx_sb = persist.tile([P, DT, L], BF16, tag="cx")

        # Input DMA rollout. The 16 DMA queues saturate at ~333GB/s
        # aggregate, so the 8MB of inputs take ~24us to land no matter how
        # configs are spread. What matters is that the critical 6MB
        # (xt/yt: the contraction dim of every projection, plus wq/wk)
        # isn't diluted by wv/wo — those 2MB are issued later, on the
        # scalar rail BEHIND the data-dependent prologue copies, so their
        # transfers can't start until the critical set has landed.
        #   SP:   xt cc0..7            (2MB)
        #   Pool: wq/wk dt0..dt3      (2MB, dt-major: dt0 lands in 0.5MB)
        #   ACT:  yt cc0..7            (2MB), then [prologue copies], wv, wo
        wv_r = wv.rearrange("(cc p) d -> p cc d", p=P)
        xt_r = xt.rearrange("(cc p) i -> p cc i", p=P)
        yt_r = yt.rearrange("(cc p) i -> p cc i", p=P)
        nc.gpsimd.dma_start(out=wq_sb[:, 0], in_=wq[0])
        nc.gpsimd.dma_start(out=wk_sb[:, 0], in_=wk[0])
        for cc in range(CC):
            nc.sync.dma_start(out=xt_sb[:, cc], in_=xt_r[:, cc])
            nc.scalar.dma_start(out=yt_sb[:, cc], in_=yt_r[:, cc])

        # ones-blocks of Vaug: columns [64,192) mod 256 of each j-chunk
        # (even heads keep V in the low half, odd heads in the high half).
        # One strided memset over half the tensor; the V halves are written
        # by the v_chain drains.
        va_ones = va_sb.rearrange("p it (q s) -> p it q s", s=2 * P)
        nc.vector.memset(va_ones[:, :, :, DH : DH + P], 1.0)

        scale = DH**-0.5

        # ---- chain emitters (each a short burst of independent PE work) ----

        def proj_chain(w_sb, t_sb, rhs_sb, dt, ic, copy_eng="vector"):
            ps = ps_acc.tile([P, NI], F32, tag="acc")
            for cc in range(CC):
                nc.tensor.matmul(
                    ps[:],
                    w_sb[:, dt, cc * P : (cc + 1) * P],
                    rhs_sb[:, cc, ic * NI : (ic + 1) * NI],
                    start=(cc == 0),
                    stop=(cc == CC - 1),
                )
            dst = t_sb[:, dt, ic * NI : (ic + 1) * NI]
            if copy_eng == "vector":
                nc.vector.tensor_copy(dst, ps[:])
            else:
                nc.scalar.copy(dst, ps[:])

        def v_chain(jt):
            ps = ps_acc.tile([P, NI], F32, tag="acc")
            for cc in range(CC):
                nc.tensor.matmul(
                    ps[:],
                    yt_sb[:, cc, jt * P : (jt + 1) * P],
                    wv_sb[:, cc, :],
                    start=(cc == 0),
                    stop=(cc == CC - 1),
                )
            va_blk = va_sb[:, jt].rearrange("p (h s) -> p h s", s=P)
            ps_blk = ps.rearrange("p (h s) -> p h s", s=DH)
            nc.vector.tensor_copy(va_blk[:, 0::2, 0:DH], ps_blk[:, 0::2, :])
            nc.vector.tensor_copy(va_blk[:, 1::2, DH:P], ps_blk[:, 1::2, :])

        # Deferred finishers: the normalize crosses engines (DVE -> gpsimd
        # partition_broadcast -> DVE); emitting the post-broadcast DVE ops
        # immediately would stall the in-order DVE stream (and the PSUM-
        # releasing copies queued behind it) on the gpsimd semaphore.
        # Instead each ctx chain queues them and the next fill slot flushes.
        deferred = []

        def flush_deferred():
            while deferred:
                deferred.pop(0)()

        def ctx_chain(h, ptile, ic, ct=None, jts=None, hold=None):
            if ptile is None:
                # fill inside the pair whose P^T this chain consumes
                ptile = st_pair.current
            dt, r0 = divmod(h * DH, P)
            lo, hi = jts or (0, IT)
            if hold:
                # continuation: accumulate into the partial held by part A
                cts = hold[0]
            else:
                if ct is None:
                    ct = ps_acc.tile([P, NI], F32, tag="acc")
                    cts = ct[:]
                else:
                    cts = ct
                if hold is not None:
                    hold.append(cts)
            for jt in range(lo, hi):
                nc.tensor.matmul(
                    cts,
                    va_sb[:, jt, h * P : (h + 1) * P],
                    ptile[:, jt, ic, h & 1],
                    start=(jt == 0),
                    stop=(jt == IT - 1),
                )
            if hi < IT:
                return
            # The 64 rowsum rows of ct are identical copies (each ones-column
            # of Vaug reproduces the row sum), so a gpsimd partition
            # broadcast of a single row moves the rowsum to the partitions
            # the ctx rows live on — no DMA round trip. The custom DVE
            # reciprocal only works at base partition 0.
            rc = small.tile([P, NI], F32, tag="rc")
            if r0 == 0:
                # ctx in rows 0:DH, rowsum copies in rows DH:P. The gpsimd
                # broadcast source must sit at partition 0 (Q7 core 0 owns
                # partitions 0:16 and does the read), so this orientation
                # has to move the rowsum down with a SBUF->SBUF DMA.
                rs = small.tile([P, NI], F32, tag="rs")
                nc.vector.tensor_copy(rs[DH:P, :], cts[DH:P, :])
                nc.gpsimd.dma_start(out=rs[0:DH, :], in_=rs[DH:P, :])

                def fin():
                    nc.vector.reciprocal_approx_fast(rc[0:DH, :], rs[0:DH, :])
                    nc.vector.tensor_mul(
                        cx_sb[0:DH, dt, ic * NI : (ic + 1) * NI],
                        cts[0:DH, :],
                        rc[0:DH, :],
                    )
            else:
                # rowsum copies in rows 0:DH, ctx in rows DH:P: reciprocal
                # of a single row at base 0 (all DH rowsum rows are
                # identical), then gpsimd partition-broadcast (the Q7 impl
                # reads the source on core 0 and write-masks partitions
                # [0, channels) absolutely, so broadcast all 128 rows).
                nc.vector.reciprocal_approx_fast(rc[0:1, :], cts[0:1, :])
                nc.gpsimd.partition_broadcast(rc[0:P, :], rc[0:1, :])

                def fin():
                    nc.vector.tensor_mul(
                        cx_sb[DH:P, dt, ic * NI : (ic + 1) * NI],
                        cts[DH:P, :],
                        rc[DH:P, :],
                    )

            deferred.append(fin)

        def po_chain(it, oc, dts, out_ap, copy_eng="vector", po=None, dma_eng=None):
            # out-projection partial over the given d-tiles
            if po is None:
                po = ps_acc.tile([P, NI], F32, tag="acc")
            for k, dt in enumerate(dts):
                nc.tensor.matmul(
                    po[:],
                    cx_sb[:, dt, it * P : (it + 1) * P],
                    wo_sb[:, dt, oc * NI : (oc + 1) * NI],
                    start=(k == 0),
                    stop=(k == len(dts) - 1),
                )
            o_st = small.tile([P, NI], BF16, tag="ost")
            if copy_eng == "vector":
                nc.vector.tensor_copy(o_st[:], po[:])
            else:
                # scalar engine is idle once the exp stream has drained
                nc.scalar.copy(o_st[:], po[:])
            out_r = out_ap.rearrange("(it p) o -> it p o", p=P)
            dma_eng = dma_eng or nc.sync
            dma_eng.dma_start(
                out=out_r[it, :, oc * NI : (oc + 1) * NI], in_=o_st[:]
            )

        # ---- ST + exp for a head pair, fill chains between steps ----

        def st_pair(hp, fills):
            # 16 steps of one wide ST tile each: step (ic, jt) computes both
            # heads' [128, 512] score blocks into the two banks of one wide
            # tile (the K=64 matmuls sit on disjoint PE row-groups AND
            # disjoint PSUM banks, so they stream concurrently), and one exp
            # drains the whole tile into the pair's P^T tensor. One wide
            # tile per step keeps the ST pipeline 2 steps deep on a 2-buf
            # pool, leaving 4 banks for the acc pool.
            dt = hp
            ptp = pt_pool.tile([P, IT, NIC, 2, NI], BF16, tag="pt")
            st_pair.current = ptp
            fills = list(fills)
            s = 0
            for ic in range(NIC):
                for jt in range(IT):
                    # fill BEFORE the step's STs: the fill chain's first
                    # LDWEIGHTS hides under the previous ST stream, and the
                    # ST LDWEIGHTS hides under the fill's last matmul (the
                    # concurrent ST pair itself offers no cover window).
                    # Dependency-wise a fill at slot s still runs after
                    # exp(s-1), same as running it after the STs.
                    if s < len(fills):
                        # pending finishers BEFORE this slot's fills, so a
                        # fill chain never re-claims an acc tile whose
                        # normalize is still queued behind the fill's own
                        # PSUM-releasing copy in the in-order DVE stream
                        pending = list(deferred)
                        deferred.clear()
                        for f in pending:
                            f()
                        for f in fills[s]:
                            f()
                    s += 1
                    stw = ps_wide.tile([P, 2 * NI], F32, tag="wide", name="stw")
                    for h_off in range(2):
                        r0 = DH * h_off
                        nc.tensor.matmul(
                            stw[:, h_off * NI : (h_off + 1) * NI],
                            kt_sb[r0 : r0 + DH, dt, jt * P : (jt + 1) * P],
                            qt_sb[r0 : r0 + DH, dt, ic * NI : (ic + 1) * NI],
                            start=True,
                            stop=True,
                        )
                    nc.scalar.activation(
                        ptp[:, jt, ic],
                        stw[:],
                        mybir.ActivationFunctionType.Exp,
                        scale=scale,
                    )
            return ptp

        # ---- schedule ----
        mk = lambda f, *a: (lambda: f(*a))

        # Prologue: all four dt0 chains (QT ic0/ic1, KT ic0/ic1) accumulate
        # per-cc in lockstep across the four acc bufs, so every xt/yt chunk
        # is consumed the moment it lands and the prologue ends right after
        # the last input chunk — instead of running four serial chains
        # after the data arrived. Drains on the scalar engine (idle until
        # the first exp); the wv/wo DMA configs queue behind these copies.
        pro = [ps_acc.tile([P, NI], F32, tag="acc", name=f"pro{k}") for k in range(4)]
        for cc in range(CC):
            st0 = cc == 0
            sp1 = cc == CC - 1
            for ic in range(NIC):
                nc.tensor.matmul(
                    pro[ic][:], wq_sb[:, 0, cc * P : (cc + 1) * P],
                    xt_sb[:, cc, ic * NI : (ic + 1) * NI], start=st0, stop=sp1,
                )
                nc.tensor.matmul(
                    pro[2 + ic][:], wk_sb[:, 0, cc * P : (cc + 1) * P],
                    yt_sb[:, cc, ic * NI : (ic + 1) * NI], start=st0, stop=sp1,
                )
        # copy order: the first ST step (ic0, jt0) reads qt-ic0 + kt-ic0
        # only — emit those first so it starts after two copies, not four
        nc.scalar.copy(qt_sb[:, 0, 0:NI], pro[0][:])
        nc.scalar.copy(kt_sb[:, 0, 0:NI], pro[2][:])
        nc.scalar.copy(qt_sb[:, 0, NI : 2 * NI], pro[1][:])
        nc.scalar.copy(kt_sb[:, 0, NI : 2 * NI], pro[3][:])

        # The non-critical 3.5MB (wq/wk dt1..3, wv, wo) is issued from
        # inside the early pair-0 fill slots on the scalar rail: each
        # config lands between two exps, and the transfers only start once
        # the critical 4.5MB has drained — none of it dilutes the startup
        # window. Consumers: QT1/KT1 fills (~+5us), v chains (~+12us),
        # QT2+/wo much later.
        d = lambda o, i: (lambda: nc.scalar.dma_start(out=o, in_=i))
        cfg = [
            d(wq_sb[:, 1], wq[1]),
            d(wk_sb[:, 1], wk[1]),
            d(wv_sb[:, 0:4], wv_r[:, 0:4]),
            d(wv_sb[:, 4:8], wv_r[:, 4:8]),
            d(wq_sb[:, 2], wq[2]),
            d(wk_sb[:, 2], wk[2]),
            d(wq_sb[:, 3], wq[3]),
            d(wk_sb[:, 3], wk[3]),
            d(wo_sb[:], wo.rearrange("(dt p) o -> p dt o", p=P)),
        ]

        # pair 0: non-critical DMA configs in the early slots, QT1/KT1
        # once their dt-blocks land (~+5us), V chains in the back half
        # (wv lands mid-phase); v6/v7 spill into pair 1's first slots.
        q1a = mk(proj_chain, wq_sb, qt_sb, xt_sb, 1, 0)
        q1b = mk(proj_chain, wq_sb, qt_sb, xt_sb, 1, 1)
        k1a = mk(proj_chain, wk_sb, kt_sb, yt_sb, 1, 0)
        k1b = mk(proj_chain, wk_sb, kt_sb, yt_sb, 1, 1)
        pt0 = st_pair(
            0,
            [
                [cfg[0]], [cfg[1]], [cfg[2]], [cfg[3]],
                [q1a, cfg[4]], [cfg[5]], [q1b, cfg[6]], [cfg[7]],
                [k1a, cfg[8]], [k1b],
                [mk(v_chain, 0)], [mk(v_chain, 1)], [mk(v_chain, 2)],
                [mk(v_chain, 3)], [mk(v_chain, 4)], [mk(v_chain, 5)],
            ],
        )

        # pair 1: last V chains, then ctx of heads 0/1 alternating with
        # QT2/KT2
        pt1 = st_pair(
            1,
            [
                [mk(v_chain, 6)], [mk(v_chain, 7)],
                [mk(ctx_chain, 0, pt0, 0)],
                [mk(proj_chain, wq_sb, qt_sb, xt_sb, 2, 0)], [],
                [mk(ctx_chain, 0, pt0, 1)],
                [mk(proj_chain, wq_sb, qt_sb, xt_sb, 2, 1)], [],
                [mk(ctx_chain, 1, pt0, 0)],
                [mk(proj_chain, wk_sb, kt_sb, yt_sb, 2, 0)], [],
                [mk(ctx_chain, 1, pt0, 1)],
                [mk(proj_chain, wk_sb, kt_sb, yt_sb, 2, 1)], [],
                [], [],
            ],
        )

        # pair 2: ctx of heads 2/3 alternating with QT3/KT3
        pt2 = st_pair(
            2,
            [
                [mk(ctx_chain, 2, pt1, 0)], [],
                [mk(proj_chain, wq_sb, qt_sb, xt_sb, 3, 0)], [],
                [mk(ctx_chain, 2, pt1, 1)], [],
                [mk(proj_chain, wq_sb, qt_sb, xt_sb, 3, 1)], [],
                [mk(ctx_chain, 3, pt1, 0)], [],
                [mk(proj_chain, wk_sb, kt_sb, yt_sb, 3, 0)], [],
                [mk(ctx_chain, 3, pt1, 1)], [],
                [mk(proj_chain, wk_sb, kt_sb, yt_sb, 3, 1)], [],
            ],
        )

        # out_a chains over dt 0..2: valid for every it-block once heads
        # 4/5 are normalized (by pair-3 slot 7). Copies alternate
        # scalar/vector (both drain engines have slack mid-phase); DMAs
        # alternate SP/Pool rails so no single rail's ~0.65us-per-config
        # serialization backlogs the kernel tail.
        poA = [
            mk(
                po_chain, it, oc, (0, 1, 2), out_a,
                ("vector", "scalar")[(2 * it + oc) % 2], None,
                (nc.sync, nc.gpsimd)[(2 * it + oc) % 2],
            )
            for it in range(IT)
            for oc in range(NIC)
        ]

        # merged out_c unit for one it-block: both oc halves into one
        # [P, L] staging tile, one 256KB DMA (halves the tail's config
        # count). In-phase units draw two acc tiles; tail units use the
        # two halves of one (by then idle) wide ST tile.
        out_cr = out_c.rearrange("(it p) o -> it p o", p=P)

        def oc_unit(it, p0=None, p1=None):
            in_phase = p0 is None
            if p0 is None:
                p0 = ps_acc.tile([P, NI], F32, tag="acc", name="ocp0")
                p1 = ps_acc.tile([P, NI], F32, tag="acc", name="ocp1")
            for oc, po in ((0, p0), (1, p1)):
                nc.tensor.matmul(
                    po,
                    cx_sb[:, 3, it * P : (it + 1) * P],
                    wo_sb[:, 3, oc * NI : (oc + 1) * NI],
                    start=True,
                    stop=True,
                )
            ot = small.tile([P, L], BF16, tag="ost2", name=f"oc{it}")
            # in-phase: both copies on DVE (the scalar engine is the exp
            # pacer there); tail: split across both drain engines
            (nc.vector.tensor_copy if in_phase else nc.scalar.copy)(
                ot[:, 0:NI], p0
            )
            nc.vector.tensor_copy(ot[:, NI : 2 * NI], p1)
            (nc.sync, nc.gpsimd)[it % 2].dma_start(out=out_cr[it], in_=ot[:])

        # pair 3: ctx of heads 4/5 first (finishers flushed by slot 7),
        # then — because the ic-outer step order finishes all ic0 exps at
        # mid-phase — ctx of heads 6/7 ic0 runs IN-phase (slots 8/9), the
        # it0..3 out_c units (which only read the ic0 columns of cx dt3)
        # drain in the last slots, and the ic1 chains of heads 6/7
        # pre-accumulate jt0..5 in slots 13/14 (their exps land by step
        # 13). Only the jt6/7 continuations + normalizes + it4..7 remain
        # for the tail.
        pt3 = st_pair(
            3,
            [
                [mk(ctx_chain, 4, pt2, 0)], [],
                [mk(ctx_chain, 4, pt2, 1)], [],
                [mk(ctx_chain, 5, pt2, 0)], [],
                [mk(ctx_chain, 5, pt2, 1)], [],
                [mk(ctx_chain, 6, None, 0)],
                [mk(ctx_chain, 7, None, 0), poA[0]],
                poA[1:3],
                poA[3:5],
                poA[5:7] + [mk(oc_unit, 0)],
                poA[7:9] + [mk(oc_unit, 1)],
                poA[9:11] + [mk(oc_unit, 2)],
                poA[11:12] + [mk(oc_unit, 3)],
            ],
        )

        # tail: ctx of heads 6/7 ic1, four out_a chains plugging the
        # normalize-latency window (h6ic1's rs DMA round trip is ~3us),
        # then the it4..7 out_c units on wide-tile halves.
        ctx_chain(6, pt3, 1)
        ctx_chain(7, pt3, 1)
        for f in poA[12:16]:
            f()
        flush_deferred()  # fins for h6ic1 and h7ic1
        for it in range(IT // 2, IT):
            pw = ps_wide.tile([P, 2 * NI], F32, tag="wide", name="po_w")
            oc_unit(it, pw[:, 0:NI], pw[:, NI : 2 * NI])

        if dbg is not None:
            nc.sync.dma_start(out=dbg[0][:], in_=qt_sb[:])
            nc.sync.dma_start(out=dbg[1][:], in_=kt_sb[:])
            nc.sync.dma_start(out=dbg[2][:], in_=va_sb[:])
            nc.sync.dma_start(out=dbg[4][:], in_=cx_sb[:])


_NC_CACHE = None


def _get_nc():
    global _NC_CACHE
    if _NC_CACHE is None:
        _NC_CACHE = _build_kernel()
    return _NC_CACHE


def kernel(x, y, Wq, Wk, Wv, Wo, _trace=False):
    bf = ml_dtypes.bfloat16
    x = np.asarray(x, np.float32)
    y = np.asarray(y, np.float32)
    xtb = [np.ascontiguousarray(np.asarray(x[b]).T).astype(bf) for b in range(B)]
    ytb = [np.ascontiguousarray(np.asarray(y[b]).T).astype(bf) for b in range(B)]
    def _dt_major(w, t):
        # [D, DL] slice -> [DT, P, CC*128]: element (dt, p, cc*128+d) =
        # w[cc*128+p, t*DL + dt*128 + d]  (proj lhsT chunks [P, 128] per
        # (dt, cc), partition dim = contraction rows)
        ws = np.asarray(w)[:, t * DL : (t + 1) * DL]          # [1024, 512]
        ws = ws.reshape(CC, P, DT, P).transpose(2, 1, 0, 3)    # [DT,P,CC,128]
        return np.ascontiguousarray(ws.reshape(DT, P, CC * P)).astype(bf)

    wqs = [_dt_major(Wq, t) for t in range(TP)]
    wks = [_dt_major(Wk, t) for t in range(TP)]
    wvs = [np.ascontiguousarray(np.asarray(Wv)[:, t * DL : (t + 1) * DL]).astype(bf) for t in range(TP)]
    wos = [np.ascontiguousarray(np.asarray(Wo)[t * DL : (t + 1) * DL, :]).astype(bf) for t in range(TP)]

    in_maps = []
    for b in range(B):
        for t in range(TP):
            in_maps.append(
                {
                    "xt": xtb[b],
                    "yt": ytb[b],
                    "wq": wqs[t],
                    "wk": wks[t],
                    "wv": wvs[t],
                    "wo": wos[t],
                }
            )

    nc = _get_nc()
    res = run_bass_kernel_spmd(
        nc, in_maps, core_ids=list(range(N_CORES)), trace=_trace
    )
    out = np.empty((B, L, U), np.float32)
    for b in range(B):
        out[b] = (
            np.asarray(res.results[2 * b]["out_a"], np.float32)
            + np.asarray(res.results[2 * b]["out_c"], np.float32)
            + np.asarray(res.results[2 * b + 1]["out_a"], np.float32)
            + np.asarray(res.results[2 * b + 1]["out_c"], np.float32)
        )
    if _trace:
        return out, res
    return out



# revision 42
# speedup vs baseline: 1.2013x; 1.2013x over previous
"""Multi-head attention (B=4, L=1024, D=1024, H=16, DH=64) on 8 TRN2 NeuronCores.

Sharding: data-parallel over batch (4) x tensor-parallel over heads (2).
Core c = 2*b + t computes, for batch b, heads [t*8, (t+1)*8):
    QT = Wq_t^T X^T, KT = Wk_t^T X^T, V = Y Wv_t        (all bf16 matmuls)
    per head: S^T = K_h Q_h^T; P^T = exp(S^T/8);
              [ctx^T; rowsum] = Vaug_h^T P^T;  ctxn = ctx / rowsum
    O_partial = ctxn^T Wo_t                              (f32, two dt-halves)
Host pre-transposes X/Y, casts to bf16, and sums the four f32 partials
(2 tensor-parallel cores x 2 dt-halves) per batch.

Engines execute their compiled instruction streams in order, so the emission
order is a hand-software-pipelined schedule: every ST (scores) step, whose exp
drain on the scalar engine is slower than the matmuls, is followed by an
independent fill chain (V projection, next d-tile QT/KT, an earlier head's
ctx, or an out-projection partial) so the tensor engine never waits for the
scalar engine to free an ST PSUM tile.

Perf notes (vs the first working version):
  - Input DMA configs are spread across four sequencers (SP/Pool/DVE/ACT);
    a single SP rail configures queues at ~0.6us each, serializing the
    input rollout and starving the PE for the first ~15us.
  - The ones-blocks of Vaug are memset with one strided op (half the data).
  - The first QT/KT drains go to the scalar engine (idle before the exps).
  - Tail: ctx tiles for the last head pair live in the (by then idle) wide
    ST PSUM pool so the out-projection chains get the full 4-slot acc pool;
    tail drains alternate scalar/vector; chain order puts both ic0 ctx
    chains first so their normalize DMA round-trips hide under ic1's PE
    work. Keeping the PE stream dense also holds it at the 2.4GHz p-state
    (it drops to 1.2GHz within ~100ns of going idle).
"""

import numpy as np
import ml_dtypes

import concourse.tile as tile
import concourse.mybir as mybir
from concourse import bacc
from concourse.bass_utils import run_bass_kernel_spmd

B, L, D, U, H = 4, 1024, 1024, 1024, 16
DH = U // H          # 64 head dim
TP = 2               # tensor-parallel ways (heads)
DL = U // TP         # 512 local units
HL = H // TP         # 8 local heads
P = 128              # partitions
NI = 512             # matmul free-dim chunk (one PSUM bank of f32)
CC = D // P          # 8 contraction chunks for projections
DT = DL // P         # 4 local d-tiles
IT = L // P          # 8 i/j tiles
NIC = L // NI        # 2 free chunks of 512
N_CORES = 8

BF16 = mybir.dt.bfloat16
F32 = mybir.dt.float32


def _build_kernel():
    nc = bacc.Bacc(
        "TRN2", target_bir_lowering=False, debug=False, num_devices=N_CORES
    )
    xt = nc.dram_tensor("xt", [D, L], BF16, kind="ExternalInput").ap()
    yt = nc.dram_tensor("yt", [D, L], BF16, kind="ExternalInput").ap()
    # wq/wk arrive dt-major: [DT, P, CC*128] (host pre-arranged) so each
    # dt-block is one contiguous 256KB DMA
    wq = nc.dram_tensor("wq", [DT, P, CC * P], BF16, kind="ExternalInput").ap()
    wk = nc.dram_tensor("wk", [DT, P, CC * P], BF16, kind="ExternalInput").ap()
    wv = nc.dram_tensor("wv", [D, DL], BF16, kind="ExternalInput").ap()
    wo = nc.dram_tensor("wo", [DL, U], BF16, kind="ExternalInput").ap()
    out_a = nc.dram_tensor("out_a", [L, U], BF16, kind="ExternalOutput").ap()
    out_c = nc.dram_tensor("out_c", [L, U], BF16, kind="ExternalOutput").ap()

    with tile.TileContext(nc) as tc:
        _mha_body(tc, out_a, out_c, xt, yt, wq, wk, wv, wo)

    nc.compile()
    return nc


def _mha_body(tc, out_a, out_c, xt, yt, wq, wk, wv, wo, dbg=None):
    nc = tc.nc
    from contextlib import ExitStack

    with ExitStack() as ctx:
        persist = ctx.enter_context(tc.tile_pool(name="persist", bufs=1))
        # P^T tiles are per-pair now; live set = current pair + previous
        # (whose ctx chains consume it)
        pt_pool = ctx.enter_context(tc.tile_pool(name="pt", bufs=2))
        # ST tiles: [P, 1024] f32 = 2 banks each; one per (jt, ic) step
        # holding BOTH heads' 512-blocks, so the two K=64 matmuls land in
        # different banks and stream concurrently on disjoint PE row groups
        ps_wide = ctx.enter_context(tc.tile_pool(name="ps_wide", bufs=2, space="PSUM"))
        # single-bank accumulators (projections, V, ctx, out-proj)
        ps_acc = ctx.enter_context(tc.tile_pool(name="ps_acc", bufs=4, space="PSUM"))
        small = ctx.enter_context(tc.tile_pool(name="small", bufs=4))

        # persistent SBUF tensors
        xt_sb = persist.tile([P, CC, L], BF16, tag="xt")
        yt_sb = persist.tile([P, CC, L], BF16, tag="yt")
        # wq/wk are dt-major (host pre-arranged [DT, P, CC*128]) so the
        # dt0 blocks needed by the first ST land after 0.5MB of weight DMA
        # instead of 2MB
        wq_sb = persist.tile([P, DT, CC * P], BF16, tag="wq")
        wk_sb = persist.tile([P, DT, CC * P], BF16, tag="wk")
        wv_sb = persist.tile([P, CC, DL], BF16, tag="wv")
        wo_sb = persist.tile([P, DT, U], BF16, tag="wo")
        qt_sb = persist.tile([P, DT, L], BF16, tag="qt")
        kt_sb = persist.tile([P, DT, L], BF16, tag="kt")
        # Vaug: per j-chunk, per head a 128-col block; even h: [V_h | ones],
        # odd h: [ones | V_h] (ctx^T lands on the head's own cx partitions)
        va_sb = persist.tile([P, IT, HL * P], BF16, tag="va")
        cx_sb = persist.tile([P, DT, L], BF16, tag="cx")

        # Input DMA rollout. The 16 DMA queues saturate at ~333GB/s
        # aggregate, so the 8MB of inputs take ~24us to land no matter how
        # configs are spread. What matters is that the critical 6MB
        # (xt/yt: the contraction dim of every projection, plus wq/wk)
        # isn't diluted by wv/wo — those 2MB are issued later, on the
        # scalar rail BEHIND the data-dependent prologue copies, so their
        # transfers can't start until the critical set has landed.
        #   SP:   xt cc0..7            (2MB)
        #   Pool: wq/wk dt0..dt3      (2MB, dt-major: dt0 lands in 0.5MB)
        #   ACT:  yt cc0..7            (2MB), then [prologue copies], wv, wo
        wv_r = wv.rearrange("(cc p) d -> p cc d", p=P)
        xt_r = xt.rearrange("(cc p) i -> p cc i", p=P)
        yt_r = yt.rearrange("(cc p) i -> p cc i", p=P)
        nc.gpsimd.dma_start(out=wq_sb[:, 0], in_=wq[0])
        nc.gpsimd.dma_start(out=wk_sb[:, 0], in_=wk[0])
        for cc in range(CC):
            nc.sync.dma_start(out=xt_sb[:, cc], in_=xt_r[:, cc])
            nc.scalar.dma_start(out=yt_sb[:, cc], in_=yt_r[:, cc])

        # ones-blocks of Vaug: columns [64,192) mod 256 of each j-chunk
        # (even heads keep V in the low half, odd heads in the high half).
        # One strided memset over half the tensor; the V halves are written
        # by the v_chain drains.
        va_ones = va_sb.rearrange("p it (q s) -> p it q s", s=2 * P)
        nc.vector.memset(va_ones[:, :, :, DH : DH + P], 1.0)

        scale = DH**-0.5

        # ---- chain emitters (each a short burst of independent PE work) ----

        def proj_chain(w_sb, t_sb, rhs_sb, dt, ic, copy_eng="vector"):
            ps = ps_acc.tile([P, NI], F32, tag="acc")
            for cc in range(CC):
                nc.tensor.matmul(
                    ps[:],
                    w_sb[:, dt, cc * P : (cc + 1) * P],
                    rhs_sb[:, cc, ic * NI : (ic + 1) * NI],
                    start=(cc == 0),
                    stop=(cc == CC - 1),
                )
            dst = t_sb[:, dt, ic * NI : (ic + 1) * NI]
            if copy_eng == "vector":
                nc.vector.tensor_copy(dst, ps[:])
            else:
                nc.scalar.copy(dst, ps[:])

        def v_chain(jt):
            ps = ps_acc.tile([P, NI], F32, tag="acc")
            for cc in range(CC):
                nc.tensor.matmul(
                    ps[:],
                    yt_sb[:, cc, jt * P : (jt + 1) * P],
                    wv_sb[:, cc, :],
                    start=(cc == 0),
                    stop=(cc == CC - 1),
                )
            va_blk = va_sb[:, jt].rearrange("p (h s) -> p h s", s=P)
            ps_blk = ps.rearrange("p (h s) -> p h s", s=DH)
            nc.vector.tensor_copy(va_blk[:, 0::2, 0:DH], ps_blk[:, 0::2, :])
            nc.vector.tensor_copy(va_blk[:, 1::2, DH:P], ps_blk[:, 1::2, :])

        # Deferred finishers: the normalize crosses engines (DVE -> gpsimd
        # partition_broadcast -> DVE); emitting the post-broadcast DVE ops
        # immediately would stall the in-order DVE stream (and the PSUM-
        # releasing copies queued behind it) on the gpsimd semaphore.
        # Instead each ctx chain queues them and the next fill slot flushes.
        deferred = []

        def flush_deferred():
            while deferred:
                deferred.pop(0)()

        def ctx_chain(h, ptile, ic, ct=None, jts=None, hold=None):
            if ptile is None:
                # fill inside the pair whose P^T this chain consumes
                ptile = st_pair.current
            dt, r0 = divmod(h * DH, P)
            lo, hi = jts or (0, IT)
            if hold:
                # continuation: accumulate into the partial held by part A
                cts = hold[0]
            else:
                if ct is None:
                    ct = ps_acc.tile([P, NI], F32, tag="acc")
                    cts = ct[:]
                else:
                    cts = ct
                if hold is not None:
                    hold.append(cts)
            for jt in range(lo, hi):
                nc.tensor.matmul(
                    cts,
                    va_sb[:, jt, h * P : (h + 1) * P],
                    ptile[:, jt, ic, h & 1],
                    start=(jt == 0),
                    stop=(jt == IT - 1),
                )
            if hi < IT:
                return
            # The 64 rowsum rows of ct are identical copies (each ones-column
            # of Vaug reproduces the row sum), so a gpsimd partition
            # broadcast of a single row moves the rowsum to the partitions
            # the ctx rows live on — no DMA round trip. The custom DVE
            # reciprocal only works at base partition 0.
            rc = small.tile([P, NI], F32, tag="rc")
            if r0 == 0:
                # ctx in rows 0:DH, rowsum copies in rows DH:P. The gpsimd
                # broadcast source must sit at partition 0 (Q7 core 0 owns
                # partitions 0:16 and does the read), so this orientation
                # has to move the rowsum down with a SBUF->SBUF DMA.
                rs = small.tile([P, NI], F32, tag="rs")
                nc.vector.tensor_copy(rs[DH:P, :], cts[DH:P, :])
                nc.gpsimd.dma_start(out=rs[0:DH, :], in_=rs[DH:P, :])

                def fin():
                    nc.vector.reciprocal_approx_fast(rc[0:DH, :], rs[0:DH, :])
                    nc.vector.tensor_mul(
                        cx_sb[0:DH, dt, ic * NI : (ic + 1) * NI],
                        cts[0:DH, :],
                        rc[0:DH, :],
                    )
            else:
                # rowsum copies in rows 0:DH, ctx in rows DH:P: reciprocal
                # of a single row at base 0 (all DH rowsum rows are
                # identical), then gpsimd partition-broadcast (the Q7 impl
                # reads the source on core 0 and write-masks partitions
                # [0, channels) absolutely, so broadcast all 128 rows).
                nc.vector.reciprocal_approx_fast(rc[0:1, :], cts[0:1, :])
                nc.gpsimd.partition_broadcast(rc[0:P, :], rc[0:1, :])

                def fin():
                    nc.vector.tensor_mul(
                        cx_sb[DH:P, dt, ic * NI : (ic + 1) * NI],
                        cts[DH:P, :],
                        rc[DH:P, :],
                    )

            deferred.append(fin)

        def po_chain(it, oc, dts, out_ap, copy_eng="vector", po=None, dma_eng=None):
            # out-projection partial over the given d-tiles
            if po is None:
                po = ps_acc.tile([P, NI], F32, tag="acc")
            for k, dt in enumerate(dts):
                nc.tensor.matmul(
                    po[:],
                    cx_sb[:, dt, it * P : (it + 1) * P],
                    wo_sb[:, dt, oc * NI : (oc + 1) * NI],
                    start=(k == 0),
                    stop=(k == len(dts) - 1),
                )
            o_st = small.tile([P, NI], BF16, tag="ost")
            if copy_eng == "vector":
                nc.vector.tensor_copy(o_st[:], po[:])
            else:
                # scalar engine is idle once the exp stream has drained
                nc.scalar.copy(o_st[:], po[:])
            out_r = out_ap.rearrange("(it p) o -> it p o", p=P)
            dma_eng = dma_eng or nc.sync
            dma_eng.dma_start(
                out=out_r[it, :, oc * NI : (oc + 1) * NI], in_=o_st[:]
            )

        # ---- ST + exp for a head pair, fill chains between steps ----

        def st_pair(hp, fills):
            # 16 steps of one wide ST tile each: step (ic, jt) computes both
            # heads' [128, 512] score blocks into the two banks of one wide
            # tile (the K=64 matmuls sit on disjoint PE row-groups AND
            # disjoint PSUM banks, so they stream concurrently), and one exp
            # drains the whole tile into the pair's P^T tensor. One wide
            # tile per step keeps the ST pipeline 2 steps deep on a 2-buf
            # pool, leaving 4 banks for the acc pool.
            dt = hp
            ptp = pt_pool.tile([P, IT, NIC, 2, NI], BF16, tag="pt")
            st_pair.current = ptp
            fills = list(fills)
            s = 0
            for ic in range(NIC):
                for jt in range(IT):
                    stw = ps_wide.tile([P, 2 * NI], F32, tag="wide", name="stw")
                    for h_off in range(2):
                        r0 = DH * h_off
                        nc.tensor.matmul(
                            stw[:, h_off * NI : (h_off + 1) * NI],
                            kt_sb[r0 : r0 + DH, dt, jt * P : (jt + 1) * P],
                            qt_sb[r0 : r0 + DH, dt, ic * NI : (ic + 1) * NI],
                            start=True,
                            stop=True,
                        )
                    # exp immediately after the STs — emitting it any later
                    # (e.g. after the fill) delays every exp by one fill,
                    # which shrinks the effective ST pipeline depth and
                    # stalls the whole PE stream on wide-tile recycling
                    nc.scalar.activation(
                        ptp[:, jt, ic],
                        stw[:],
                        mybir.ActivationFunctionType.Exp,
                        scale=scale,
                    )
                    if s < len(fills):
                        # pending finishers BEFORE this slot's fills, so a
                        # fill chain never re-claims an acc tile whose
                        # normalize is still queued behind the fill's own
                        # PSUM-releasing copy in the in-order DVE stream
                        pending = list(deferred)
                        deferred.clear()
                        for f in pending:
                            f()
                        for f in fills[s]:
                            f()
                    s += 1
            return ptp

        # ---- schedule ----
        mk = lambda f, *a: (lambda: f(*a))

        # Prologue: all four dt0 chains (QT ic0/ic1, KT ic0/ic1) accumulate
        # per-cc in lockstep across the four acc bufs, so every xt/yt chunk
        # is consumed the moment it lands and the prologue ends right after
        # the last input chunk — instead of running four serial chains
        # after the data arrived. Drains on the scalar engine (idle until
        # the first exp); the wv/wo DMA configs queue behind these copies.
        pro = [ps_acc.tile([P, NI], F32, tag="acc", name=f"pro{k}") for k in range(4)]
        for cc in range(CC):
            st0 = cc == 0
            sp1 = cc == CC - 1
            for ic in range(NIC):
                nc.tensor.matmul(
                    pro[ic][:], wq_sb[:, 0, cc * P : (cc + 1) * P],
                    xt_sb[:, cc, ic * NI : (ic + 1) * NI], start=st0, stop=sp1,
                )
                nc.tensor.matmul(
                    pro[2 + ic][:], wk_sb[:, 0, cc * P : (cc + 1) * P],
                    yt_sb[:, cc, ic * NI : (ic + 1) * NI], start=st0, stop=sp1,
                )
        # copy order: the first ST step (ic0, jt0) reads qt-ic0 + kt-ic0
        # only — emit those first so it starts after two copies, not four
        nc.scalar.copy(qt_sb[:, 0, 0:NI], pro[0][:])
        nc.scalar.copy(kt_sb[:, 0, 0:NI], pro[2][:])
        nc.scalar.copy(qt_sb[:, 0, NI : 2 * NI], pro[1][:])
        nc.scalar.copy(kt_sb[:, 0, NI : 2 * NI], pro[3][:])

        # The non-critical 3.5MB (wq/wk dt1..3, wv, wo) is issued from
        # inside the early pair-0 fill slots on the scalar rail: each
        # config lands between two exps, and the transfers only start once
        # the critical 4.5MB has drained — none of it dilutes the startup
        # window. Consumers: QT1/KT1 fills (~+5us), v chains (~+12us),
        # QT2+/wo much later.
        d = lambda o, i: (lambda: nc.scalar.dma_start(out=o, in_=i))
        cfg = [
            d(wq_sb[:, 1], wq[1]),
            d(wk_sb[:, 1], wk[1]),
            d(wv_sb[:, 0:4], wv_r[:, 0:4]),
            d(wv_sb[:, 4:8], wv_r[:, 4:8]),
            d(wq_sb[:, 2], wq[2]),
            d(wk_sb[:, 2], wk[2]),
            d(wq_sb[:, 3], wq[3]),
            d(wk_sb[:, 3], wk[3]),
            d(wo_sb[:], wo.rearrange("(dt p) o -> p dt o", p=P)),
        ]

        # pair 0: non-critical DMA configs in the early slots, QT1/KT1
        # once their dt-blocks land (~+5us), V chains in the back half
        # (wv lands mid-phase); v6/v7 spill into pair 1's first slots.
        q1a = mk(proj_chain, wq_sb, qt_sb, xt_sb, 1, 0)
        q1b = mk(proj_chain, wq_sb, qt_sb, xt_sb, 1, 1)
        k1a = mk(proj_chain, wk_sb, kt_sb, yt_sb, 1, 0)
        k1b = mk(proj_chain, wk_sb, kt_sb, yt_sb, 1, 1)
        pt0 = st_pair(
            0,
            [
                [cfg[0]], [cfg[1]], [cfg[2]], [cfg[3]],
                [q1a, cfg[4]], [cfg[5]], [q1b, cfg[6]], [cfg[7]],
                [k1a, cfg[8]], [k1b],
                [mk(v_chain, 0)], [mk(v_chain, 1)], [mk(v_chain, 2)],
                [mk(v_chain, 3)], [mk(v_chain, 4)], [mk(v_chain, 5)],
            ],
        )

        # pair 1: last V chains, then ctx of heads 0/1 alternating with
        # QT2/KT2
        pt1 = st_pair(
            1,
            [
                [mk(v_chain, 6)], [mk(v_chain, 7)],
                [mk(ctx_chain, 0, pt0, 0)],
                [mk(proj_chain, wq_sb, qt_sb, xt_sb, 2, 0)], [],
                [mk(ctx_chain, 0, pt0, 1)],
                [mk(proj_chain, wq_sb, qt_sb, xt_sb, 2, 1)], [],
                [mk(ctx_chain, 1, pt0, 0)],
                [mk(proj_chain, wk_sb, kt_sb, yt_sb, 2, 0)], [],
                [mk(ctx_chain, 1, pt0, 1)],
                [mk(proj_chain, wk_sb, kt_sb, yt_sb, 2, 1)], [],
                [], [],
            ],
        )

        # pair 2: ctx of heads 2/3 alternating with QT3/KT3
        pt2 = st_pair(
            2,
            [
                [mk(ctx_chain, 2, pt1, 0)], [],
                [mk(proj_chain, wq_sb, qt_sb, xt_sb, 3, 0)], [],
                [mk(ctx_chain, 2, pt1, 1)], [],
                [mk(proj_chain, wq_sb, qt_sb, xt_sb, 3, 1)], [],
                [mk(ctx_chain, 3, pt1, 0)], [],
                [mk(proj_chain, wk_sb, kt_sb, yt_sb, 3, 0)], [],
                [mk(ctx_chain, 3, pt1, 1)], [],
                [mk(proj_chain, wk_sb, kt_sb, yt_sb, 3, 1)], [],
            ],
        )

        # out_a chains over dt 0..2: valid for every it-block once heads
        # 4/5 are normalized (by pair-3 slot 7). Copies alternate
        # scalar/vector (both drain engines have slack mid-phase); DMAs
        # alternate SP/Pool rails so no single rail's ~0.65us-per-config
        # serialization backlogs the kernel tail.
        poA = [
            mk(
                po_chain, it, oc, (0, 1, 2), out_a,
                ("vector", "scalar")[(2 * it + oc) % 2], None,
                (nc.sync, nc.gpsimd)[(2 * it + oc) % 2],
            )
            for it in range(IT)
            for oc in range(NIC)
        ]

        # merged out_c unit for one it-block: both oc halves into one
        # [P, L] staging tile, one 256KB DMA (halves the tail's config
        # count). In-phase units draw two acc tiles; tail units use the
        # two halves of one (by then idle) wide ST tile.
        out_cr = out_c.rearrange("(it p) o -> it p o", p=P)

        def oc_unit(it, p0=None, p1=None):
            in_phase = p0 is None
            if p0 is None:
                p0 = ps_acc.tile([P, NI], F32, tag="acc", name="ocp0")
                p1 = ps_acc.tile([P, NI], F32, tag="acc", name="ocp1")
            for oc, po in ((0, p0), (1, p1)):
                nc.tensor.matmul(
                    po,
                    cx_sb[:, 3, it * P : (it + 1) * P],
                    wo_sb[:, 3, oc * NI : (oc + 1) * NI],
                    start=True,
                    stop=True,
                )
            ot = small.tile([P, L], BF16, tag="ost2", name=f"oc{it}")
            # in-phase: both copies on DVE (the scalar engine is the exp
            # pacer there); tail: split across both drain engines
            (nc.vector.tensor_copy if in_phase else nc.scalar.copy)(
                ot[:, 0:NI], p0
            )
            nc.vector.tensor_copy(ot[:, NI : 2 * NI], p1)
            (nc.sync, nc.gpsimd)[it % 2].dma_start(out=out_cr[it], in_=ot[:])

        # pair 3: ctx of heads 4/5 first (finishers flushed by slot 7),
        # then — because the ic-outer step order finishes all ic0 exps at
        # mid-phase — ctx of heads 6/7 ic0 runs IN-phase (slots 8/9), the
        # it0..3 out_c units (which only read the ic0 columns of cx dt3)
        # drain in the last slots, and the ic1 chains of heads 6/7
        # pre-accumulate jt0..5 in slots 13/14 (their exps land by step
        # 13). Only the jt6/7 continuations + normalizes + it4..7 remain
        # for the tail.
        pt3 = st_pair(
            3,
            [
                [mk(ctx_chain, 4, pt2, 0)], [],
                [mk(ctx_chain, 4, pt2, 1)], [],
                [mk(ctx_chain, 5, pt2, 0)], [],
                [mk(ctx_chain, 5, pt2, 1)], [],
                [mk(ctx_chain, 6, None, 0)],
                [mk(ctx_chain, 7, None, 0), poA[0]],
                poA[1:3],
                poA[3:5],
                poA[5:7] + [mk(oc_unit, 0)],
                poA[7:9] + [mk(oc_unit, 1)],
                poA[9:11] + [mk(oc_unit, 2)],
                poA[11:12] + [mk(oc_unit, 3)],
            ],
        )

        # tail: ctx of heads 6/7 ic1, four out_a chains plugging the
        # normalize-latency window (h6ic1's rs DMA round trip is ~3us),
        # then the it4..7 out_c units on wide-tile halves.
        ctx_chain(6, pt3, 1)
        ctx_chain(7, pt3, 1)
        for f in poA[12:16]:
            f()
        flush_deferred()  # fins for h6ic1 and h7ic1
        for it in range(IT // 2, IT):
            pw = ps_wide.tile([P, 2 * NI], F32, tag="wide", name="po_w")
            oc_unit(it, pw[:, 0:NI], pw[:, NI : 2 * NI])

        if dbg is not None:
            nc.sync.dma_start(out=dbg[0][:], in_=qt_sb[:])
            nc.sync.dma_start(out=dbg[1][:], in_=kt_sb[:])
            nc.sync.dma_start(out=dbg[2][:], in_=va_sb[:])
            nc.sync.dma_start(out=dbg[4][:], in_=cx_sb[:])


_NC_CACHE = None


def _get_nc():
    global _NC_CACHE
    if _NC_CACHE is None:
        _NC_CACHE = _build_kernel()
    return _NC_CACHE


def kernel(x, y, Wq, Wk, Wv, Wo, _trace=False):
    bf = ml_dtypes.bfloat16
    x = np.asarray(x, np.float32)
    y = np.asarray(y, np.float32)
    xtb = [np.ascontiguousarray(np.asarray(x[b]).T).astype(bf) for b in range(B)]
    ytb = [np.ascontiguousarray(np.asarray(y[b]).T).astype(bf) for b in range(B)]
    def _dt_major(w, t):
        # [D, DL] slice -> [DT, P, CC*128]: element (dt, p, cc*128+d) =
        # w[cc*128+p, t*DL + dt*128 + d]  (proj lhsT chunks [P, 128] per
        # (dt, cc), partition dim = contraction rows)
        ws = np.asarray(w)[:, t * DL : (t + 1) * DL]          # [1024, 512]
        ws = ws.reshape(CC, P, DT, P).transpose(2, 1, 0, 3)    # [DT,P,CC,128]
        return np.ascontiguousarray(ws.reshape(DT, P, CC * P)).astype(bf)

    wqs = [_dt_major(Wq, t) for t in range(TP)]
    wks = [_dt_major(Wk, t) for t in range(TP)]
    wvs = [np.ascontiguousarray(np.asarray(Wv)[:, t * DL : (t + 1) * DL]).astype(bf) for t in range(TP)]
    wos = [np.ascontiguousarray(np.asarray(Wo)[t * DL : (t + 1) * DL, :]).astype(bf) for t in range(TP)]

    in_maps = []
    for b in range(B):
        for t in range(TP):
            in_maps.append(
                {
                    "xt": xtb[b],
                    "yt": ytb[b],
                    "wq": wqs[t],
                    "wk": wks[t],
                    "wv": wvs[t],
                    "wo": wos[t],
                }
            )

    nc = _get_nc()
    res = run_bass_kernel_spmd(
        nc, in_maps, core_ids=list(range(N_CORES)), trace=_trace
    )
    out = np.empty((B, L, U), np.float32)
    for b in range(B):
        out[b] = (
            np.asarray(res.results[2 * b]["out_a"], np.float32)
            + np.asarray(res.results[2 * b]["out_c"], np.float32)
            + np.asarray(res.results[2 * b + 1]["out_a"], np.float32)
            + np.asarray(res.results[2 * b + 1]["out_c"], np.float32)
        )
    if _trace:
        return out, res
    return out



# revision 52
# speedup vs baseline: 1.2036x; 1.0019x over previous
"""Multi-head attention (B=4, L=1024, D=1024, H=16, DH=64) on 8 TRN2 NeuronCores.

Sharding: data-parallel over batch (4) x tensor-parallel over heads (2).
Core c = 2*b + t computes, for batch b, heads [t*8, (t+1)*8):
    QT = Wq_t^T X^T, KT = Wk_t^T X^T, V = Y Wv_t        (all bf16 matmuls)
    per head: S^T = K_h Q_h^T; P^T = exp(S^T/8);
              [ctx^T; rowsum] = Vaug_h^T P^T;  ctxn = ctx / rowsum
    O_partial = ctxn^T Wo_t                              (f32, two dt-halves)
Host pre-transposes X/Y, casts to bf16, and sums the four f32 partials
(2 tensor-parallel cores x 2 dt-halves) per batch.

Engines execute their compiled instruction streams in order, so the emission
order is a hand-software-pipelined schedule: every ST (scores) step, whose exp
drain on the scalar engine is slower than the matmuls, is followed by an
independent fill chain (V projection, next d-tile QT/KT, an earlier head's
ctx, or an out-projection partial) so the tensor engine never waits for the
scalar engine to free an ST PSUM tile.

Perf notes (154.6us baseline -> ~138us):
  - Scores concurrency: each (jt, ic) step computes BOTH heads' K=64
    score blocks into the two banks of ONE wide PSUM tile; the matmuls
    sit on disjoint PE row-groups (partitions 0:64 / 64:128) and
    disjoint banks, so they stream concurrently in the array (measured
    ~2x: ST avg 152ns vs 213 serial). One exp drains the whole tile.
  - Startup is DMA-bound: the 16 DMA queues saturate at ~333GB/s, so
    only the critical 4.5MB (xt, yt, wq/wk dt0) is issued up front;
    wq/wk are host-rearranged dt-major so dt0 arrives in the first
    0.5MB. The prologue runs all four dt0 projection chains per-cc in
    lockstep with the arrivals. The non-critical 3.5MB (wq/wk dt1-3,
    wv, wo) is issued from inside early pair-0 fill slots on the
    scalar rail — each config lands between two exps, and none of its
    transfers dilute the critical window (first ST at ~24.5us vs 31.4).
  - exp emission must stay immediately after its ST matmuls: delaying
    it by one fill shrinks the 2-deep wide-tile pipeline and stalls the
    whole PE stream (+24us measured).
  - The ic-outer step order finishes all ic0 exps mid-phase, so pair 3
    runs ctx h6/h7 ic0 and the it0-3 out_c drains IN-phase; only the
    ic1 chains and it4-7 drains remain after the last exp (tail 12us).
  - out DMAs: out_c merges both oc halves into one 256KB transfer per
    it-block; configs alternate SP/Pool rails (one rail's 0.65us/config
    serialization otherwise backlogs the tail by ~10us).
  - PSUM budget: 2 wide ST tiles (4 banks) + 4 acc banks. Shrinking
    the acc pool below 4 (for a 3rd wide tile) loses more in fill-chain
    stalls than the deeper ST pipe gains — measured +3 to +20us.
"""

import numpy as np
import ml_dtypes

import concourse.tile as tile
import concourse.mybir as mybir
from concourse import bacc
from concourse.bass_utils import run_bass_kernel_spmd

B, L, D, U, H = 4, 1024, 1024, 1024, 16
DH = U // H          # 64 head dim
TP = 2               # tensor-parallel ways (heads)
DL = U // TP         # 512 local units
HL = H // TP         # 8 local heads
P = 128              # partitions
NI = 512             # matmul free-dim chunk (one PSUM bank of f32)
CC = D // P          # 8 contraction chunks for projections
DT = DL // P         # 4 local d-tiles
IT = L // P          # 8 i/j tiles
NIC = L // NI        # 2 free chunks of 512
N_CORES = 8

BF16 = mybir.dt.bfloat16
F32 = mybir.dt.float32


def _build_kernel():
    nc = bacc.Bacc(
        "TRN2", target_bir_lowering=False, debug=False, num_devices=N_CORES
    )
    xt = nc.dram_tensor("xt", [D, L], BF16, kind="ExternalInput").ap()
    yt = nc.dram_tensor("yt", [D, L], BF16, kind="ExternalInput").ap()
    # wq/wk arrive dt-major: [DT, P, CC*128] (host pre-arranged) so each
    # dt-block is one contiguous 256KB DMA
    wq = nc.dram_tensor("wq", [DT, P, CC * P], BF16, kind="ExternalInput").ap()
    wk = nc.dram_tensor("wk", [DT, P, CC * P], BF16, kind="ExternalInput").ap()
    wv = nc.dram_tensor("wv", [D, DL], BF16, kind="ExternalInput").ap()
    wo = nc.dram_tensor("wo", [DL, U], BF16, kind="ExternalInput").ap()
    out_a = nc.dram_tensor("out_a", [L, U], BF16, kind="ExternalOutput").ap()
    out_c = nc.dram_tensor("out_c", [L, U], BF16, kind="ExternalOutput").ap()

    with tile.TileContext(nc) as tc:
        _mha_body(tc, out_a, out_c, xt, yt, wq, wk, wv, wo)

    nc.compile()
    return nc


def _mha_body(tc, out_a, out_c, xt, yt, wq, wk, wv, wo, dbg=None):
    nc = tc.nc
    from contextlib import ExitStack

    with ExitStack() as ctx:
        persist = ctx.enter_context(tc.tile_pool(name="persist", bufs=1))
        # P^T tiles are per-pair now; live set = current pair + previous
        # (whose ctx chains consume it)
        pt_pool = ctx.enter_context(tc.tile_pool(name="pt", bufs=2))
        # ST tiles: [P, 1024] f32 = 2 banks each; one per (jt, ic) step
        # holding BOTH heads' 512-blocks, so the two K=64 matmuls land in
        # different banks and stream concurrently on disjoint PE row groups
        ps_wide = ctx.enter_context(tc.tile_pool(name="ps_wide", bufs=2, space="PSUM"))
        # single-bank accumulators (projections, V, ctx, out-proj)
        ps_acc = ctx.enter_context(tc.tile_pool(name="ps_acc", bufs=4, space="PSUM"))
        small = ctx.enter_context(tc.tile_pool(name="small", bufs=4))

        # persistent SBUF tensors
        xt_sb = persist.tile([P, CC, L], BF16, tag="xt")
        yt_sb = persist.tile([P, CC, L], BF16, tag="yt")
        # wq/wk are dt-major (host pre-arranged [DT, P, CC*128]) so the
        # dt0 blocks needed by the first ST land after 0.5MB of weight DMA
        # instead of 2MB
        wq_sb = persist.tile([P, DT, CC * P], BF16, tag="wq")
        wk_sb = persist.tile([P, DT, CC * P], BF16, tag="wk")
        wv_sb = persist.tile([P, CC, DL], BF16, tag="wv")
        wo_sb = persist.tile([P, DT, U], BF16, tag="wo")
        qt_sb = persist.tile([P, DT, L], BF16, tag="qt")
        kt_sb = persist.tile([P, DT, L], BF16, tag="kt")
        # Vaug: per j-chunk, per head a 128-col block; even h: [V_h | ones],
        # odd h: [ones | V_h] (ctx^T lands on the head's own cx partitions)
        va_sb = persist.tile([P, IT, HL * P], BF16, tag="va")
        cx_sb = persist.tile([P, DT, L], BF16, tag="cx")

        # Input DMA rollout. The 16 DMA queues saturate at ~333GB/s
        # aggregate, so the 8MB of inputs take ~24us to land no matter how
        # configs are spread. What matters is that the critical 6MB
        # (xt/yt: the contraction dim of every projection, plus wq/wk)
        # isn't diluted by wv/wo — those 2MB are issued later, on the
        # scalar rail BEHIND the data-dependent prologue copies, so their
        # transfers can't start until the critical set has landed.
        #   SP:   xt cc0..7            (2MB)
        #   Pool: wq/wk dt0..dt3      (2MB, dt-major: dt0 lands in 0.5MB)
        #   ACT:  yt cc0..7            (2MB), then [prologue copies], wv, wo
        wv_r = wv.rearrange("(cc p) d -> p cc d", p=P)
        xt_r = xt.rearrange("(cc p) i -> p cc i", p=P)
        yt_r = yt.rearrange("(cc p) i -> p cc i", p=P)
        nc.gpsimd.dma_start(out=wq_sb[:, 0], in_=wq[0])
        nc.gpsimd.dma_start(out=wk_sb[:, 0], in_=wk[0])
        for cc in range(CC):
            nc.sync.dma_start(out=xt_sb[:, cc], in_=xt_r[:, cc])
            nc.scalar.dma_start(out=yt_sb[:, cc], in_=yt_r[:, cc])

        # ones-blocks of Vaug: columns [64,192) mod 256 of each j-chunk
        # (even heads keep V in the low half, odd heads in the high half).
        # One strided memset over half the tensor; the V halves are written
        # by the v_chain drains.
        va_ones = va_sb.rearrange("p it (q s) -> p it q s", s=2 * P)
        nc.vector.memset(va_ones[:, :, :, DH : DH + P], 1.0)

        scale = DH**-0.5

        # ---- chain emitters (each a short burst of independent PE work) ----

        def proj_chain(w_sb, t_sb, rhs_sb, dt, ic, copy_eng="vector"):
            ps = ps_acc.tile([P, NI], F32, tag="acc")
            for cc in range(CC):
                nc.tensor.matmul(
                    ps[:],
                    w_sb[:, dt, cc * P : (cc + 1) * P],
                    rhs_sb[:, cc, ic * NI : (ic + 1) * NI],
                    start=(cc == 0),
                    stop=(cc == CC - 1),
                )
            dst = t_sb[:, dt, ic * NI : (ic + 1) * NI]
            if copy_eng == "vector":
                nc.vector.tensor_copy(dst, ps[:])
            else:
                nc.scalar.copy(dst, ps[:])

        def v_chain(jt):
            ps = ps_acc.tile([P, NI], F32, tag="acc")
            for cc in range(CC):
                nc.tensor.matmul(
                    ps[:],
                    yt_sb[:, cc, jt * P : (jt + 1) * P],
                    wv_sb[:, cc, :],
                    start=(cc == 0),
                    stop=(cc == CC - 1),
                )
            va_blk = va_sb[:, jt].rearrange("p (h s) -> p h s", s=P)
            ps_blk = ps.rearrange("p (h s) -> p h s", s=DH)
            nc.vector.tensor_copy(va_blk[:, 0::2, 0:DH], ps_blk[:, 0::2, :])
            nc.vector.tensor_copy(va_blk[:, 1::2, DH:P], ps_blk[:, 1::2, :])

        # Deferred finishers: the normalize crosses engines (DVE -> gpsimd
        # partition_broadcast -> DVE); emitting the post-broadcast DVE ops
        # immediately would stall the in-order DVE stream (and the PSUM-
        # releasing copies queued behind it) on the gpsimd semaphore.
        # Instead each ctx chain queues them and the next fill slot flushes.
        deferred = []

        def flush_deferred():
            while deferred:
                deferred.pop(0)()

        def ctx_chain(h, ptile, ic, ct=None, jts=None, hold=None):
            if ptile is None:
                # fill inside the pair whose P^T this chain consumes
                ptile = st_pair.current
            dt, r0 = divmod(h * DH, P)
            lo, hi = jts or (0, IT)
            if hold:
                # continuation: accumulate into the partial held by part A
                cts = hold[0]
            else:
                if ct is None:
                    ct = ps_acc.tile([P, NI], F32, tag="acc")
                    cts = ct[:]
                else:
                    cts = ct
                if hold is not None:
                    hold.append(cts)
            for jt in range(lo, hi):
                nc.tensor.matmul(
                    cts,
                    va_sb[:, jt, h * P : (h + 1) * P],
                    ptile[:, jt, ic, h & 1],
                    start=(jt == 0),
                    stop=(jt == IT - 1),
                )
            if hi < IT:
                return
            # The 64 rowsum rows of ct are identical copies (each ones-column
            # of Vaug reproduces the row sum), so a gpsimd partition
            # broadcast of a single row moves the rowsum to the partitions
            # the ctx rows live on — no DMA round trip. The custom DVE
            # reciprocal only works at base partition 0.
            rc = small.tile([P, NI], F32, tag="rc")
            if r0 == 0:
                # ctx in rows 0:DH, rowsum copies in rows DH:P. The gpsimd
                # broadcast source must sit at partition 0 (Q7 core 0 owns
                # partitions 0:16 and does the read), so this orientation
                # has to move the rowsum down with a SBUF->SBUF DMA.
                rs = small.tile([P, NI], F32, tag="rs")
                nc.vector.tensor_copy(rs[DH:P, :], cts[DH:P, :])
                nc.gpsimd.dma_start(out=rs[0:DH, :], in_=rs[DH:P, :])

                def fin():
                    nc.vector.reciprocal_approx_fast(rc[0:DH, :], rs[0:DH, :])
                    nc.vector.tensor_mul(
                        cx_sb[0:DH, dt, ic * NI : (ic + 1) * NI],
                        cts[0:DH, :],
                        rc[0:DH, :],
                    )
            else:
                # rowsum copies in rows 0:DH, ctx in rows DH:P: reciprocal
                # of a single row at base 0 (all DH rowsum rows are
                # identical), then gpsimd partition-broadcast (the Q7 impl
                # reads the source on core 0 and write-masks partitions
                # [0, channels) absolutely, so broadcast all 128 rows).
                nc.vector.reciprocal_approx_fast(rc[0:1, :], cts[0:1, :])
                nc.gpsimd.partition_broadcast(rc[0:P, :], rc[0:1, :])

                def fin():
                    nc.vector.tensor_mul(
                        cx_sb[DH:P, dt, ic * NI : (ic + 1) * NI],
                        cts[DH:P, :],
                        rc[DH:P, :],
                    )

            deferred.append(fin)

        def po_chain(it, oc, dts, out_ap, copy_eng="vector", po=None, dma_eng=None):
            # out-projection partial over the given d-tiles
            if po is None:
                po = ps_acc.tile([P, NI], F32, tag="acc")
            for k, dt in enumerate(dts):
                nc.tensor.matmul(
                    po[:],
                    cx_sb[:, dt, it * P : (it + 1) * P],
                    wo_sb[:, dt, oc * NI : (oc + 1) * NI],
                    start=(k == 0),
                    stop=(k == len(dts) - 1),
                )
            o_st = small.tile([P, NI], BF16, tag="ost")
            if copy_eng == "vector":
                nc.vector.tensor_copy(o_st[:], po[:])
            else:
                # scalar engine is idle once the exp stream has drained
                nc.scalar.copy(o_st[:], po[:])
            out_r = out_ap.rearrange("(it p) o -> it p o", p=P)
            dma_eng = dma_eng or nc.sync
            dma_eng.dma_start(
                out=out_r[it, :, oc * NI : (oc + 1) * NI], in_=o_st[:]
            )

        # ---- ST + exp for a head pair, fill chains between steps ----

        def st_pair(hp, fills):
            # 16 steps of one wide ST tile each: step (ic, jt) computes both
            # heads' [128, 512] score blocks into the two banks of one wide
            # tile (the K=64 matmuls sit on disjoint PE row-groups AND
            # disjoint PSUM banks, so they stream concurrently), and one exp
            # drains the whole tile into the pair's P^T tensor. One wide
            # tile per step keeps the ST pipeline 2 steps deep on a 2-buf
            # pool, leaving 4 banks for the acc pool.
            dt = hp
            ptp = pt_pool.tile([P, IT, NIC, 2, NI], BF16, tag="pt")
            st_pair.current = ptp
            fills = list(fills)
            s = 0
            for ic in range(NIC):
                for jt in range(IT):
                    stw = ps_wide.tile([P, 2 * NI], F32, tag="wide", name="stw")
                    for h_off in range(2):
                        r0 = DH * h_off
                        nc.tensor.matmul(
                            stw[:, h_off * NI : (h_off + 1) * NI],
                            kt_sb[r0 : r0 + DH, dt, jt * P : (jt + 1) * P],
                            qt_sb[r0 : r0 + DH, dt, ic * NI : (ic + 1) * NI],
                            start=True,
                            stop=True,
                        )
                    # exp immediately after the STs — emitting it any later
                    # (e.g. after the fill) delays every exp by one fill,
                    # which shrinks the effective ST pipeline depth and
                    # stalls the whole PE stream on wide-tile recycling
                    nc.scalar.activation(
                        ptp[:, jt, ic],
                        stw[:],
                        mybir.ActivationFunctionType.Exp,
                        scale=scale,
                    )
                    if s < len(fills):
                        # pending finishers BEFORE this slot's fills, so a
                        # fill chain never re-claims an acc tile whose
                        # normalize is still queued behind the fill's own
                        # PSUM-releasing copy in the in-order DVE stream
                        pending = list(deferred)
                        deferred.clear()
                        for f in pending:
                            f()
                        for f in fills[s]:
                            f()
                    s += 1
            return ptp

        # ---- schedule ----
        mk = lambda f, *a: (lambda: f(*a))

        # Prologue: all four dt0 chains (QT ic0/ic1, KT ic0/ic1) accumulate
        # per-cc in lockstep across the four acc bufs, so every xt/yt chunk
        # is consumed the moment it lands and the prologue ends right after
        # the last input chunk — instead of running four serial chains
        # after the data arrived. Drains on the scalar engine (idle until
        # the first exp); the wv/wo DMA configs queue behind these copies.
        # ~8 dummy matmuls on scratch data while the first inputs are in
        # flight: they keep the PE busy through the HAM activity window so
        # the prologue and pair 0 run at 2.4GHz instead of cold 1.2GHz
        scratch = persist.tile([P, NI], BF16, tag="scratch")
        nc.vector.memset(scratch[:], 0.0)
        warm_ps = ps_acc.tile([P, NI], F32, tag="acc", name="warm")
        for _ in range(8):
            nc.tensor.matmul(warm_ps[:], scratch[:, 0:P], scratch[:], start=True, stop=True)

        # two accumulators from the acc pool, two on an idle wide tile's
        # halves (the 2-buf acc pool cannot hold four live chains)
        pro = [
            ps_acc.tile([P, NI], F32, tag="acc", name="pro0"),
            ps_acc.tile([P, NI], F32, tag="acc", name="pro1"),
        ]
        prow = ps_wide.tile([P, 2 * NI], F32, tag="wide", name="prow")
        pro += [prow[:, 0:NI], prow[:, NI : 2 * NI]]
        for cc in range(CC):
            st0 = cc == 0
            sp1 = cc == CC - 1
            for ic in range(NIC):
                nc.tensor.matmul(
                    pro[ic][:], wq_sb[:, 0, cc * P : (cc + 1) * P],
                    xt_sb[:, cc, ic * NI : (ic + 1) * NI], start=st0, stop=sp1,
                )
                nc.tensor.matmul(
                    pro[2 + ic][:], wk_sb[:, 0, cc * P : (cc + 1) * P],
                    yt_sb[:, cc, ic * NI : (ic + 1) * NI], start=st0, stop=sp1,
                )
        # copy order: the first ST step (ic0, jt0) reads qt-ic0 + kt-ic0
        # only — emit those first so it starts after two copies, not four
        nc.scalar.copy(qt_sb[:, 0, 0:NI], pro[0][:])
        nc.scalar.copy(kt_sb[:, 0, 0:NI], pro[2][:])
        nc.scalar.copy(qt_sb[:, 0, NI : 2 * NI], pro[1][:])
        nc.scalar.copy(kt_sb[:, 0, NI : 2 * NI], pro[3][:])

        # The non-critical 3.5MB (wq/wk dt1..3, wv, wo) is issued from
        # inside the early pair-0 fill slots on the scalar rail: each
        # config lands between two exps, and the transfers only start once
        # the critical 4.5MB has drained — none of it dilutes the startup
        # window. Consumers: QT1/KT1 fills (~+5us), v chains (~+12us),
        # QT2+/wo much later.
        d = lambda o, i: (lambda: nc.scalar.dma_start(out=o, in_=i))
        cfg = [
            d(wq_sb[:, 1], wq[1]),
            d(wk_sb[:, 1], wk[1]),
            d(wv_sb[:, 0:4], wv_r[:, 0:4]),
            d(wv_sb[:, 4:8], wv_r[:, 4:8]),
            d(wq_sb[:, 2], wq[2]),
            d(wk_sb[:, 2], wk[2]),
            d(wq_sb[:, 3], wq[3]),
            d(wk_sb[:, 3], wk[3]),
            d(wo_sb[:], wo.rearrange("(dt p) o -> p dt o", p=P)),
        ]

        # pair 0: non-critical DMA configs in the early slots, QT1/KT1
        # once their dt-blocks land (~+5us), V chains in the back half
        # (wv lands mid-phase); v6/v7 spill into pair 1's first slots.
        q1a = mk(proj_chain, wq_sb, qt_sb, xt_sb, 1, 0)
        q1b = mk(proj_chain, wq_sb, qt_sb, xt_sb, 1, 1)
        k1a = mk(proj_chain, wk_sb, kt_sb, yt_sb, 1, 0)
        k1b = mk(proj_chain, wk_sb, kt_sb, yt_sb, 1, 1)
        pt0 = st_pair(
            0,
            [
                [cfg[0]], [cfg[1]], [cfg[2]], [cfg[3]],
                [q1a, cfg[4]], [cfg[5]], [q1b, cfg[6]], [cfg[7]],
                [k1a, cfg[8]], [k1b],
                [mk(v_chain, 0)], [mk(v_chain, 1)], [mk(v_chain, 2)],
                [mk(v_chain, 3)], [mk(v_chain, 4)], [mk(v_chain, 5)],
            ],
        )

        # pair 1: last V chains, then ctx of heads 0/1 alternating with
        # QT2/KT2
        # chains spread over all 16 slots (trailing empty slots leave the
        # PE idling at exp pace); KT-ic0 early enough for the next pair's
        # first steps, QT-ic1/KT-ic1 only needed by its steps 8 / 4
        pt1 = st_pair(
            1,
            [
                [mk(v_chain, 6)], [mk(v_chain, 7)],
                [mk(ctx_chain, 0, pt0, 0)], [],
                [mk(ctx_chain, 0, pt0, 1)], [],
                [mk(proj_chain, wq_sb, qt_sb, xt_sb, 2, 0)], [],
                [mk(ctx_chain, 1, pt0, 0)], [],
                [mk(proj_chain, wk_sb, kt_sb, yt_sb, 2, 0)], [],
                [mk(ctx_chain, 1, pt0, 1)], [],
                [mk(proj_chain, wq_sb, qt_sb, xt_sb, 2, 1)],
                [mk(proj_chain, wk_sb, kt_sb, yt_sb, 2, 1)],
            ],
        )

        # pair 2: ctx of heads 2/3 alternating with QT3/KT3
        pt2 = st_pair(
            2,
            [
                [mk(ctx_chain, 2, pt1, 0)], [],
                [mk(ctx_chain, 2, pt1, 1)], [],
                [mk(proj_chain, wq_sb, qt_sb, xt_sb, 3, 0)], [],
                [mk(ctx_chain, 3, pt1, 0)], [],
                [mk(proj_chain, wk_sb, kt_sb, yt_sb, 3, 0)], [],
                [mk(ctx_chain, 3, pt1, 1)], [],
                [], [],
                [mk(proj_chain, wq_sb, qt_sb, xt_sb, 3, 1)],
                [mk(proj_chain, wk_sb, kt_sb, yt_sb, 3, 1)],
            ],
        )

        # out_a chains over dt 0..2: valid for every it-block once heads
        # 4/5 are normalized (by pair-3 slot 7). Copies alternate
        # scalar/vector (both drain engines have slack mid-phase); DMAs
        # alternate SP/Pool rails so no single rail's ~0.65us-per-config
        # serialization backlogs the kernel tail.
        poA = [
            mk(
                po_chain, it, oc, (0, 1, 2), out_a,
                ("vector", "scalar")[(2 * it + oc) % 2], None,
                (nc.sync, nc.gpsimd)[(2 * it + oc) % 2],
            )
            for it in range(IT)
            for oc in range(NIC)
        ]

        # merged out_c unit for one it-block: both oc halves into one
        # [P, L] staging tile, one 256KB DMA (halves the tail's config
        # count). In-phase units draw two acc tiles; tail units use the
        # two halves of one (by then idle) wide ST tile.
        out_cr = out_c.rearrange("(it p) o -> it p o", p=P)

        def oc_unit(it, p0=None, p1=None):
            in_phase = p0 is None
            if p0 is None:
                p0 = ps_acc.tile([P, NI], F32, tag="acc", name="ocp0")
                p1 = ps_acc.tile([P, NI], F32, tag="acc", name="ocp1")
            for oc, po in ((0, p0), (1, p1)):
                nc.tensor.matmul(
                    po,
                    cx_sb[:, 3, it * P : (it + 1) * P],
                    wo_sb[:, 3, oc * NI : (oc + 1) * NI],
                    start=True,
                    stop=True,
                )
            ot = small.tile([P, L], BF16, tag="ost2", name=f"oc{it}")
            # in-phase: both copies on DVE (the scalar engine is the exp
            # pacer there); tail: split across both drain engines
            (nc.vector.tensor_copy if in_phase else nc.scalar.copy)(
                ot[:, 0:NI], p0
            )
            nc.vector.tensor_copy(ot[:, NI : 2 * NI], p1)
            (nc.sync, nc.gpsimd)[it % 2].dma_start(out=out_cr[it], in_=ot[:])

        # pair 3: ctx of heads 4/5 first (finishers flushed by slot 7),
        # then — because the ic-outer step order finishes all ic0 exps at
        # mid-phase — ctx of heads 6/7 ic0 runs IN-phase (slots 8/9), the
        # it0..3 out_c units (which only read the ic0 columns of cx dt3)
        # drain in the last slots, and the ic1 chains of heads 6/7
        # pre-accumulate jt0..5 in slots 13/14 (their exps land by step
        # 13). Only the jt6/7 continuations + normalizes + it4..7 remain
        # for the tail.
        pt3 = st_pair(
            3,
            [
                [mk(ctx_chain, 4, pt2, 0)], [],
                [mk(ctx_chain, 4, pt2, 1)], [],
                [mk(ctx_chain, 5, pt2, 0)], [],
                [mk(ctx_chain, 5, pt2, 1)], [],
                [mk(ctx_chain, 6, None, 0)],
                [mk(ctx_chain, 7, None, 0), poA[0]],
                poA[1:3],
                poA[3:5],
                poA[5:7] + [mk(oc_unit, 0)],
                poA[7:9] + [mk(oc_unit, 1)],
                poA[9:11] + [mk(oc_unit, 2)],
                poA[11:12] + [mk(oc_unit, 3)],
            ],
        )

        # tail: ctx of heads 6/7 ic1, four out_a chains plugging the
        # normalize-latency window (h6ic1's rs DMA round trip is ~3us),
        # then the it4..7 out_c units on wide-tile halves.
        ctx_chain(6, pt3, 1)
        ctx_chain(7, pt3, 1)
        for f in poA[12:16]:
            f()
        flush_deferred()  # fins for h6ic1 and h7ic1
        for it in range(IT // 2, IT):
            pw = ps_wide.tile([P, 2 * NI], F32, tag="wide", name="po_w")
            oc_unit(it, pw[:, 0:NI], pw[:, NI : 2 * NI])

        if dbg is not None:
            nc.sync.dma_start(out=dbg[0][:], in_=qt_sb[:])
            nc.sync.dma_start(out=dbg[1][:], in_=kt_sb[:])
            nc.sync.dma_start(out=dbg[2][:], in_=va_sb[:])
            nc.sync.dma_start(out=dbg[4][:], in_=cx_sb[:])


_NC_CACHE = None


def _get_nc():
    global _NC_CACHE
    if _NC_CACHE is None:
        _NC_CACHE = _build_kernel()
    return _NC_CACHE


def kernel(x, y, Wq, Wk, Wv, Wo, _trace=False):
    bf = ml_dtypes.bfloat16
    x = np.asarray(x, np.float32)
    y = np.asarray(y, np.float32)
    xtb = [np.ascontiguousarray(np.asarray(x[b]).T).astype(bf) for b in range(B)]
    ytb = [np.ascontiguousarray(np.asarray(y[b]).T).astype(bf) for b in range(B)]
    def _dt_major(w, t):
        # [D, DL] slice -> [DT, P, CC*128]: element (dt, p, cc*128+d) =
        # w[cc*128+p, t*DL + dt*128 + d]  (proj lhsT chunks [P, 128] per
        # (dt, cc), partition dim = contraction rows)
        ws = np.asarray(w)[:, t * DL : (t + 1) * DL]          # [1024, 512]
        ws = ws.reshape(CC, P, DT, P).transpose(2, 1, 0, 3)    # [DT,P,CC,128]
        return np.ascontiguousarray(ws.reshape(DT, P, CC * P)).astype(bf)

    wqs = [_dt_major(Wq, t) for t in range(TP)]
    wks = [_dt_major(Wk, t) for t in range(TP)]
    wvs = [np.ascontiguousarray(np.asarray(Wv)[:, t * DL : (t + 1) * DL]).astype(bf) for t in range(TP)]
    wos = [np.ascontiguousarray(np.asarray(Wo)[t * DL : (t + 1) * DL, :]).astype(bf) for t in range(TP)]

    in_maps = []
    for b in range(B):
        for t in range(TP):
            in_maps.append(
                {
                    "xt": xtb[b],
                    "yt": ytb[b],
                    "wq": wqs[t],
                    "wk": wks[t],
                    "wv": wvs[t],
                    "wo": wos[t],
                }
            )

    nc = _get_nc()
    res = run_bass_kernel_spmd(
        nc, in_maps, core_ids=list(range(N_CORES)), trace=_trace
    )
    out = np.empty((B, L, U), np.float32)
    for b in range(B):
        out[b] = (
            np.asarray(res.results[2 * b]["out_a"], np.float32)
            + np.asarray(res.results[2 * b]["out_c"], np.float32)
            + np.asarray(res.results[2 * b + 1]["out_a"], np.float32)
            + np.asarray(res.results[2 * b + 1]["out_c"], np.float32)
        )
    if _trace:
        return out, res
    return out

